# revision 32
# baseline (speedup 1.0000x reference)
"""Trainium2 Bass kernel for Bahdanau-style additive self-attention.

Reference computation (B=4, L=512, D=512, U=64):
    q = x @ Wt; k = x @ Wx                       [B, L, U]
    h = tanh(q[:, :, None, :] + k[:, None, :, :] + bh)       [B, L, L, U]
    e = exp(sigmoid(h . Wa + ba))                [B, L, L]
    a = e / (sum_j e + 1e-7)                     (mask is all-ones per spec)
    v = a @ x                                    [B, L, D]

Sharding: 8 cores, core c handles batch item b = c // 2 and query rows
[256 * (c % 2), ...+256).  Fully data-parallel, no collectives.  Host-side
layout prep (no arithmetic): rows of each core's x shard are rolled so its
query rows are rows 0..255 (attention sums over all keys, so key order is
irrelevant); x is also passed transposed (xT) so no on-device transpose of
x is needed; Wt/Wx are passed pre-chunked [128, 4, 64] for contiguous DMA;
ba is passed replicated [128, 1].

Per-core dataflow (ScalarE tanh throughput is the hard floor: 8.39M tanh
elements / 128 lanes / 1.2 GHz = 54.6 us):
  * qT = Wt^T x^T [64, 256] and kT-stacked = [Wx|Wx]^T x^T [128, 512] on
    PE (bf16 single-pass matmuls; fp32 matmuls cost two HI/LO passes).
  * K2 [128, 512] fp32 = kT stacked twice (2-query packing along the
    partition axis).  Qp [128, 128] fp32: column t = [qT[:, 2t] + bh ;
    qT[:, 2t+1] + bh].
  * main loop over blocks of G pairs (warmup [2,2,4,8] then G=12):
    VectorE tensor_scalar builds zb[:, j*512:...] = K2 + Qp[:, t] (the
    per-partition-scalar add); ONE ScalarE TANH over [128, G*512] fp32->
    bf16 amortizes the ~222-cycle ACT overhead; then G accumulating bf16
    matvecs with the sliding-window stationary WSLIDE[:, 128-2lt:256-2lt]
    (Wa at (rows 0:64, col 128) and (rows 64:128, col 129)) place pair
    lt's two score rows at PSUM partitions (2lt, 2lt+1): 64 matvecs build
    a dense [128, 512] fp32 score tile in one PSUM bank.
  * epilogue per score tile: sigmoid(z) = .5 + .5*tanh(z/2) ->
    w = tanh(.5 z + .5 ba); E = exp(.5 w + .5) -> bf16 with accum_out
    rowsums (tanh/exp live in one ACT table set: zero table switches);
    r = 1/(rowsum + eps) on VectorE reciprocal.
  * v = E @ x via PE-transposed bf16 E chunks against bf16 x chunks,
    fp32 PSUM accumulate; the 1/rowsum scale is folded into the ScalarE
    PSUM->SBUF copy (activation Copy with per-partition scale); DMA out.
"""

import os
import sys

import numpy as np

for _p in ("/root/.axon_site", "/root/.axon_site/_ro/trn_rl_repo",
           "/root/.axon_site/_ro/pypackages", "/opt/trn_rl_repo"):
    if os.path.isdir(_p) and _p not in sys.path:
        sys.path.append(_p)

B, L, D, U = 4, 512, 512, 64
P = 128
N_CORES = 8
IH = L // 2          # 256 query rows per core
NPAIR = IH // 2      # 128 packed query pairs per core
EPS = 1e-7


def build_kernel():
    import concourse.tile as tile
    from concourse import bacc, mybir
    from concourse.masks import make_identity

    fp32 = mybir.dt.float32
    bf16 = mybir.dt.bfloat16
    AF = mybir.ActivationFunctionType
    nc = bacc.Bacc()

    x_ext = nc.declare_dram_parameter("x", [L, D], fp32, isOutput=False)
    xt_ext = nc.declare_dram_parameter("xT", [D, L], fp32, isOutput=False)
    wt_ext = nc.declare_dram_parameter("Wt", [P, 4, U], fp32, isOutput=False)
    wx_ext = nc.declare_dram_parameter("Wx", [P, 4, U], fp32, isOutput=False)
    bh_ext = nc.declare_dram_parameter("bh", [U], fp32, isOutput=False)
    wa_ext = nc.declare_dram_parameter("Wa", [U, 1], fp32, isOutput=False)
    ba_ext = nc.declare_dram_parameter("ba", [P, 1], fp32, isOutput=False)
    out_ext = nc.declare_dram_parameter("out", [IH, D], fp32, isOutput=True)

    with tile.TileContext(nc) as tc:
        with (
            tc.tile_pool(name="const", bufs=1) as const,
            tc.tile_pool(name="work", bufs=3) as work,
            tc.tile_pool(name="tanh", bufs=3) as tanhp,
            tc.tile_pool(name="zpool", bufs=4) as zpool,
            tc.tile_pool(name="psum", bufs=4, space="PSUM") as psum,
            tc.tile_pool(name="psum_s", bufs=2, space="PSUM") as psum_s,
            tc.tile_pool(name="psum_v", bufs=2, space="PSUM") as psum_v,
        ):
            # ---- DMA enqueues first so transfers start ASAP -----------------
            # xT chunks on 3 queues (critical path: projections need them)
            xt_engines = [nc.sync, nc.scalar, nc.gpsimd, nc.sync]
            xT_sb = []
            for dc in range(4):
                xtc = const.tile([P, L], fp32, tag=f"xt{dc}")
                xt_engines[dc].dma_start(xtc[:], xt_ext.ap()[dc * P:(dc + 1) * P, :])
                xT_sb.append(xtc)
            wx_sb = const.tile([P, 4, U], fp32)       # host pre-chunked [p, c, u]
            nc.scalar.dma_start(wx_sb[:], wx_ext.ap())
            wt_sb = const.tile([P, 4, U], fp32)
            nc.gpsimd.dma_start(wt_sb[:], wt_ext.ap())
            bh_sb = const.tile([U, 1], fp32)
            nc.sync.dma_start(bh_sb[:], bh_ext.ap()[:, None])
            ba_sb = const.tile([P, 1], fp32)          # ba replicated host-side
            nc.sync.dma_start(ba_sb[:], ba_ext.ap())
            wa_sb = const.tile([U, 1], fp32)
            nc.scalar.dma_start(wa_sb[:], wa_ext.ap())
            # x only feeds the v matmul (~60us in) -> load last
            x_sb = []
            for jc in range(4):
                xc = const.tile([P, D], fp32, tag=f"x{jc}")
                xt_engines[jc].dma_start(xc[:], x_ext.ap()[jc * P:(jc + 1) * P, :])
                x_sb.append(xc)

            # ---- constants; dummy tanh early hides ACT_TABLE_LOAD -----------
            half = const.tile([P, 1], fp32)
            nc.vector.memset(half[:], 0.5)
            dummy = const.tile([P, 1], fp32)
            nc.scalar.activation(dummy[:], half[:], AF.Tanh)
            ident_bf = const.tile([P, P], bf16)
            make_identity(nc, ident_bf)

            # ---- bf16 casts: xT (projection path) first ---------------------
            xT = []
            for dc in range(4):
                xtb = const.tile([P, L], bf16, tag=f"xtb{dc}")
                nc.vector.tensor_copy(out=xtb[:], in_=xT_sb[dc][:])
                xT.append(xtb)
            wt_bf = const.tile([P, 4, U], bf16)
            nc.vector.tensor_copy(out=wt_bf[:], in_=wt_sb[:])
            # doubled stationary [Wx | Wx]: kT comes out already stacked 2x
            wx2_bf = const.tile([P, 4, 2 * U], bf16)
            nc.vector.tensor_copy(out=wx2_bf[:, :, 0:U], in_=wx_sb[:])
            nc.vector.tensor_copy(out=wx2_bf[:, :, U:2 * U], in_=wx_sb[:])

            # ---- projections: qT first (qp overlaps the kT chain) -----------
            qT_ps = psum.tile([U, IH], fp32, tag="scratch")
            for dc in range(4):
                nc.tensor.matmul(qT_ps[:], lhsT=wt_bf[:, dc],
                                 rhs=xT[dc][:, 0:IH],
                                 start=(dc == 0), stop=(dc == 3))
            kT_ps = psum.tile([P, L], fp32, tag="scratch")
            for dc in range(4):
                nc.tensor.matmul(kT_ps[:], lhsT=wx2_bf[:, dc], rhs=xT[dc][:],
                                 start=(dc == 0), stop=(dc == 3))

            # Qp column t packs queries (2t, 2t+1) -> natural partition order
            qp = const.tile([P, NPAIR], fp32)
            qT_r = qT_ps.rearrange("u (t two) -> u two t", two=2)
            nc.vector.tensor_scalar(qp[0:U, :], qT_r[:, 0], bh_sb[:],
                                    None, mybir.AluOpType.add)
            nc.vector.tensor_scalar(qp[U:2 * U, :], qT_r[:, 1], bh_sb[:],
                                    None, mybir.AluOpType.add)
            k2 = const.tile([P, L], fp32)             # kT stacked twice
            nc.scalar.copy(k2[:], kT_ps[:])

            # ---- non-critical constants -------------------------------------
            wslide = const.tile([P, 2 * P], bf16)
            nc.vector.memset(wslide[:], 0.0)
            nc.vector.tensor_copy(out=wslide[0:U, P:P + 1], in_=wa_sb[:])
            nc.vector.tensor_copy(out=wslide[U:2 * U, P + 1:P + 2], in_=wa_sb[:])
            ba_half = const.tile([P, 1], fp32)
            nc.vector.tensor_scalar_mul(ba_half[:], ba_sb[:], 0.5)
            x_bf = const.tile([P, 4, D], bf16)        # bf16 x for the v matmul
            for jc in range(4):
                nc.vector.tensor_copy(out=x_bf[:, jc], in_=x_sb[jc][:])

            # ---- main loop: small warmup blocks, then G=16 steady ----------
            BLOCKS0 = [2, 2, 4, 8] + [12] * 4        # first group (fast ramp)
            BLOCKS1 = [12] * 4 + [8, 8]              # small last block: short tail
            for g in range(2):
                s_ps = psum_s.tile([P, L], fp32)
                lt = 0
                for gsz in (BLOCKS0 if g == 0 else BLOCKS1):
                    zb = zpool.tile([P, gsz * L], fp32, tag="zb")
                    for j in range(gsz):
                        t = g * 64 + lt + j
                        nc.vector.tensor_scalar_add(
                            zb[:, j * L:(j + 1) * L], k2[:], qp[:, t:t + 1])
                    tt = tanhp.tile([P, gsz * L], bf16)
                    nc.scalar.activation(tt[:], zb[:], AF.Tanh)
                    for j in range(gsz):
                        nc.tensor.matmul(
                            s_ps[:],
                            lhsT=wslide[:, P - 2 * (lt + j):2 * P - 2 * (lt + j)],
                            rhs=tt[:, j * L:(j + 1) * L],
                            start=(lt + j == 0), stop=(lt + j == 63))
                    lt += gsz

                # ---- epilogue: sigmoid via tanh, exp(+rowsum), normalize ---
                w_sb = work.tile([P, L], fp32, tag="w")
                nc.scalar.activation(w_sb[:], s_ps[:], AF.Tanh,
                                     bias=ba_half[:], scale=0.5)
                e_bf = work.tile([P, L], bf16, tag="e")
                rowsum = work.tile([P, 1], fp32, tag="rs")
                nc.scalar.activation(e_bf[:], w_sb[:], AF.Exp,
                                     bias=half[:], scale=0.5,
                                     accum_out=rowsum[:])
                recip = work.tile([P, 1], fp32, tag="rc")
                nc.vector.tensor_scalar_add(recip[:], rowsum[:], EPS)
                nc.vector.reciprocal(recip[:], recip[:])

                # ---- v_raw = E @ x (bf16), then v = v_raw * recip ----------
                v_ps = psum_v.tile([P, D], fp32)
                for jc in range(4):
                    at_ps = psum.tile([P, P], bf16, tag="scratch")
                    nc.tensor.transpose(at_ps[:], e_bf[:, jc * P:(jc + 1) * P],
                                        ident_bf[:])
                    at_sb = work.tile([P, P], bf16, tag="at_sb")
                    nc.vector.tensor_copy(out=at_sb[:], in_=at_ps[:])
                    nc.tensor.matmul(v_ps[:], lhsT=at_sb[:], rhs=x_bf[:, jc],
                                     start=(jc == 0), stop=(jc == 3))
                v_sb = work.tile([P, D], fp32, tag="v")
                nc.scalar.activation(v_sb[:], v_ps[:], AF.Copy, bias=0.0,
                                     scale=recip[:])
                nc.sync.dma_start(out_ext.ap()[g * P:g * P + 64, :],
                                  v_sb[0:64, :])
                nc.sync.dma_start(out_ext.ap()[g * P + 64:(g + 1) * P, :],
                                   v_sb[64:P, :])

    return nc


_NC_CACHE = None


def make_in_maps(x, Wt, Wx, bh, Wa, ba):
    x = np.ascontiguousarray(np.asarray(x, dtype=np.float32))
    Wt = np.ascontiguousarray(
        np.asarray(Wt, dtype=np.float32).reshape(4, P, U).transpose(1, 0, 2))
    Wx = np.ascontiguousarray(
        np.asarray(Wx, dtype=np.float32).reshape(4, P, U).transpose(1, 0, 2))
    bh = np.ascontiguousarray(np.asarray(bh, dtype=np.float32))
    Wa = np.ascontiguousarray(np.asarray(Wa, dtype=np.float32)).reshape(U, 1)
    ba = np.ascontiguousarray(
        np.full((P, 1), np.asarray(ba, dtype=np.float32).reshape(()), np.float32))

    in_maps = []
    for c in range(N_CORES):
        b, ih = c // 2, c % 2
        # Attention sums over all keys j, so key order is irrelevant; roll the
        # rows so this core's 256 query rows are always rows 0..255 of its x.
        xb = x[b] if ih == 0 else np.roll(x[b], -IH, axis=0)
        in_maps.append({
            "x": np.ascontiguousarray(xb),
            "xT": np.ascontiguousarray(xb.T),
            "Wt": Wt, "Wx": Wx, "bh": bh, "Wa": Wa, "ba": ba,
        })
    return in_maps


def assemble_out(results):
    out = np.empty((B, L, D), dtype=np.float32)
    for c in range(N_CORES):
        b, ih = c // 2, c % 2
        out[b, ih * IH:(ih + 1) * IH, :] = results[c]["out"]
    return out


def kernel(x, mask, Wt, Wx, bh, Wa, ba):
    """Full inputs -> full output [B, L, D]. Shards over 8 NeuronCores."""
    global _NC_CACHE
    from concourse.bass_utils import run_bass_kernel_spmd

    if _NC_CACHE is None:
        _NC_CACHE = build_kernel()
        _NC_CACHE.finalize()
    nc = _NC_CACHE

    in_maps = make_in_maps(x, Wt, Wx, bh, Wa, ba)
    res = run_bass_kernel_spmd(nc, in_maps, core_ids=list(range(N_CORES)))
    return assemble_out(res.results)


if __name__ == "__main__":
    rng = np.random.default_rng(0)
    x = rng.standard_normal((B, L, D), dtype=np.float32)
    out = kernel(x, np.ones((B, L), bool),
                 rng.standard_normal((D, U), dtype=np.float32) * 0.05,
                 rng.standard_normal((D, U), dtype=np.float32) * 0.05,
                 np.zeros(U, np.float32),
                 rng.standard_normal((U, 1), dtype=np.float32) * 0.17,
                 np.zeros(1, np.float32))
    print(out.shape, out.dtype)


# revision 33
# speedup vs baseline: 1.0045x; 1.0045x over previous
"""Trainium2 Bass kernel for Bahdanau-style additive self-attention.

Reference computation (B=4, L=512, D=512, U=64):
    q = x @ Wt; k = x @ Wx                       [B, L, U]
    h = tanh(q[:, :, None, :] + k[:, None, :, :] + bh)       [B, L, L, U]
    e = exp(sigmoid(h . Wa + ba))                [B, L, L]
    a = e / (sum_j e + 1e-7)                     (mask is all-ones per spec)
    v = a @ x                                    [B, L, D]

Sharding: 8 cores, core c handles batch item b = c // 2 and query rows
[256 * (c % 2), ...+256).  Fully data-parallel, no collectives.  Host-side
layout prep (no arithmetic): rows of each core's x shard are rolled so its
query rows are rows 0..255 (attention sums over all keys, so key order is
irrelevant); x is also passed transposed (xT) so no on-device transpose of
x is needed; Wt/Wx are passed pre-chunked [128, 4, 64] for contiguous DMA;
ba is passed replicated [128, 1].

Per-core dataflow (ScalarE tanh throughput is the hard floor: 8.39M tanh
elements / 128 lanes / 1.2 GHz = 54.6 us):
  * qT = Wt^T x^T [64, 256] and kT-stacked = [Wx|Wx]^T x^T [128, 512] on
    PE (bf16 single-pass matmuls; fp32 matmuls cost two HI/LO passes).
  * K2 [128, 512] fp32 = kT stacked twice (2-query packing along the
    partition axis).  Qp [128, 128] fp32: column t = [qT[:, 2t] + bh ;
    qT[:, 2t+1] + bh].
  * main loop over blocks of G pairs (warmup [2,2,4,8] then G=12):
    VectorE tensor_scalar builds zb[:, j*512:...] = K2 + Qp[:, t] (the
    per-partition-scalar add); ONE ScalarE TANH over [128, G*512] fp32->
    bf16 amortizes the ~222-cycle ACT overhead; then G accumulating bf16
    matvecs with the sliding-window stationary WSLIDE[:, 128-2lt:256-2lt]
    (Wa at (rows 0:64, col 128) and (rows 64:128, col 129)) place pair
    lt's two score rows at PSUM partitions (2lt, 2lt+1): 64 matvecs build
    a dense [128, 512] fp32 score tile in one PSUM bank.
  * epilogue per score tile: sigmoid(z) = .5 + .5*tanh(z/2) ->
    w = tanh(.5 z + .5 ba); E = exp(.5 w + .5) -> bf16 with accum_out
    rowsums (tanh/exp live in one ACT table set: zero table switches);
    r = 1/(rowsum + eps) on VectorE reciprocal.
  * v = E @ x via PE-transposed bf16 E chunks against bf16 x chunks,
    fp32 PSUM accumulate; the 1/rowsum scale is folded into the ScalarE
    PSUM->SBUF copy (activation Copy with per-partition scale); DMA out.
"""

import os
import sys

import numpy as np

for _p in ("/root/.axon_site", "/root/.axon_site/_ro/trn_rl_repo",
           "/root/.axon_site/_ro/pypackages", "/opt/trn_rl_repo"):
    if os.path.isdir(_p) and _p not in sys.path:
        sys.path.append(_p)

B, L, D, U = 4, 512, 512, 64
P = 128
N_CORES = 8
IH = L // 2          # 256 query rows per core
NPAIR = IH // 2      # 128 packed query pairs per core
EPS = 1e-7


def build_kernel():
    import concourse.tile as tile
    from concourse import bacc, mybir
    from concourse.masks import make_identity

    fp32 = mybir.dt.float32
    bf16 = mybir.dt.bfloat16
    AF = mybir.ActivationFunctionType
    nc = bacc.Bacc()

    x_ext = nc.declare_dram_parameter("x", [L, D], fp32, isOutput=False)
    xt_ext = nc.declare_dram_parameter("xT", [D, L], fp32, isOutput=False)
    wt_ext = nc.declare_dram_parameter("Wt", [P, 4, U], fp32, isOutput=False)
    wx_ext = nc.declare_dram_parameter("Wx", [P, 4, U], fp32, isOutput=False)
    bh_ext = nc.declare_dram_parameter("bh", [U], fp32, isOutput=False)
    wa_ext = nc.declare_dram_parameter("Wa", [U, 1], fp32, isOutput=False)
    ba_ext = nc.declare_dram_parameter("ba", [P, 1], fp32, isOutput=False)
    out_ext = nc.declare_dram_parameter("out", [IH, D], fp32, isOutput=True)

    with tile.TileContext(nc) as tc:
        with (
            tc.tile_pool(name="const", bufs=1) as const,
            tc.tile_pool(name="work", bufs=3) as work,
            tc.tile_pool(name="tanh", bufs=3) as tanhp,
            tc.tile_pool(name="psum", bufs=4, space="PSUM") as psum,
            tc.tile_pool(name="psum_s", bufs=2, space="PSUM") as psum_s,
            tc.tile_pool(name="psum_v", bufs=2, space="PSUM") as psum_v,
        ):
            # ---- DMA enqueues first so transfers start ASAP -----------------
            # xT chunks on 3 queues (critical path: projections need them)
            xt_engines = [nc.sync, nc.scalar, nc.gpsimd, nc.sync]
            xT_sb = []
            for dc in range(4):
                xtc = const.tile([P, L], fp32, tag=f"xt{dc}")
                xt_engines[dc].dma_start(xtc[:], xt_ext.ap()[dc * P:(dc + 1) * P, :])
                xT_sb.append(xtc)
            wx_sb = const.tile([P, 4, U], fp32)       # host pre-chunked [p, c, u]
            nc.scalar.dma_start(wx_sb[:], wx_ext.ap())
            wt_sb = const.tile([P, 4, U], fp32)
            nc.gpsimd.dma_start(wt_sb[:], wt_ext.ap())
            bh_sb = const.tile([U, 1], fp32)
            nc.sync.dma_start(bh_sb[:], bh_ext.ap()[:, None])
            ba_sb = const.tile([P, 1], fp32)          # ba replicated host-side
            nc.sync.dma_start(ba_sb[:], ba_ext.ap())
            wa_sb = const.tile([U, 1], fp32)
            nc.scalar.dma_start(wa_sb[:], wa_ext.ap())
            # x only feeds the v matmul (~60us in) -> load last
            x_sb = []
            for jc in range(4):
                xc = const.tile([P, D], fp32, tag=f"x{jc}")
                xt_engines[jc].dma_start(xc[:], x_ext.ap()[jc * P:(jc + 1) * P, :])
                x_sb.append(xc)

            # ---- constants; dummy tanh early hides ACT_TABLE_LOAD -----------
            half = const.tile([P, 1], fp32)
            nc.vector.memset(half[:], 0.5)
            dummy = const.tile([P, 1], fp32)
            nc.scalar.activation(dummy[:], half[:], AF.Tanh)
            ident_bf = const.tile([P, P], bf16)
            make_identity(nc, ident_bf)

            # ---- bf16 casts: xT (projection path) first ---------------------
            xT = []
            for dc in range(4):
                xtb = const.tile([P, L], bf16, tag=f"xtb{dc}")
                nc.vector.tensor_copy(out=xtb[:], in_=xT_sb[dc][:])
                xT.append(xtb)
            wt_bf = const.tile([P, 4, U], bf16)
            nc.vector.tensor_copy(out=wt_bf[:], in_=wt_sb[:])
            # doubled stationary [Wx | Wx]: kT comes out already stacked 2x
            wx2_bf = const.tile([P, 4, 2 * U], bf16)
            nc.vector.tensor_copy(out=wx2_bf[:, :, 0:U], in_=wx_sb[:])
            nc.vector.tensor_copy(out=wx2_bf[:, :, U:2 * U], in_=wx_sb[:])

            # ---- projections: qT first (qp overlaps the kT chain) -----------
            qT_ps = psum.tile([U, IH], fp32, tag="scratch")
            for dc in range(4):
                nc.tensor.matmul(qT_ps[:], lhsT=wt_bf[:, dc],
                                 rhs=xT[dc][:, 0:IH],
                                 start=(dc == 0), stop=(dc == 3))
            kT_ps = psum.tile([P, L], fp32, tag="scratch")
            for dc in range(4):
                nc.tensor.matmul(kT_ps[:], lhsT=wx2_bf[:, dc], rhs=xT[dc][:],
                                 start=(dc == 0), stop=(dc == 3))

            # Qp column t packs queries (2t, 2t+1) -> natural partition order
            qp = const.tile([P, NPAIR], fp32)
            qT_r = qT_ps.rearrange("u (t two) -> u two t", two=2)
            nc.vector.tensor_scalar(qp[0:U, :], qT_r[:, 0], bh_sb[:],
                                    None, mybir.AluOpType.add)
            nc.vector.tensor_scalar(qp[U:2 * U, :], qT_r[:, 1], bh_sb[:],
                                    None, mybir.AluOpType.add)
            k2 = const.tile([P, L], fp32)             # kT stacked twice
            nc.scalar.copy(k2[:], kT_ps[:])

            # ---- non-critical constants -------------------------------------
            wslide = const.tile([P, 2 * P], bf16)
            nc.vector.memset(wslide[:], 0.0)
            nc.vector.tensor_copy(out=wslide[0:U, P:P + 1], in_=wa_sb[:])
            nc.vector.tensor_copy(out=wslide[U:2 * U, P + 1:P + 2], in_=wa_sb[:])
            ba_half = const.tile([P, 1], fp32)
            nc.vector.tensor_scalar_mul(ba_half[:], ba_sb[:], 0.5)
            x_bf = const.tile([P, 4, D], bf16)        # bf16 x for the v matmul
            for jc in range(4):
                nc.vector.tensor_copy(out=x_bf[:, jc], in_=x_sb[jc][:])

            # ---- main loop: small warmup blocks, then G=16 steady ----------
            BLOCKS0 = [2, 2, 4, 8] + [12] * 4        # first group (fast ramp)
            BLOCKS1 = [12] * 4 + [8, 8]              # small last block: short tail
            for g in range(2):
                s_ps = psum_s.tile([P, L], fp32)
                lt = 0
                for gsz in (BLOCKS0 if g == 0 else BLOCKS1):
                    zb = work.tile([P, gsz * L], fp32, tag="zb")
                    for j in range(gsz):
                        t = g * 64 + lt + j
                        nc.vector.tensor_scalar_add(
                            zb[:, j * L:(j + 1) * L], k2[:], qp[:, t:t + 1])
                    tt = tanhp.tile([P, gsz * L], bf16)
                    nc.scalar.activation(tt[:], zb[:], AF.Tanh)
                    for j in range(gsz):
                        nc.tensor.matmul(
                            s_ps[:],
                            lhsT=wslide[:, P - 2 * (lt + j):2 * P - 2 * (lt + j)],
                            rhs=tt[:, j * L:(j + 1) * L],
                            start=(lt + j == 0), stop=(lt + j == 63))
                    lt += gsz

                # ---- epilogue: sigmoid via tanh, exp(+rowsum), normalize ---
                w_sb = work.tile([P, L], fp32, tag="w")
                nc.scalar.activation(w_sb[:], s_ps[:], AF.Tanh,
                                     bias=ba_half[:], scale=0.5)
                e_bf = work.tile([P, L], bf16, tag="e")
                rowsum = work.tile([P, 1], fp32, tag="rs")
                nc.scalar.activation(e_bf[:], w_sb[:], AF.Exp,
                                     bias=half[:], scale=0.5,
                                     accum_out=rowsum[:])
                recip = work.tile([P, 1], fp32, tag="rc")
                nc.vector.tensor_scalar_add(recip[:], rowsum[:], EPS)
                nc.vector.reciprocal(recip[:], recip[:])

                # ---- v_raw = E @ x (bf16), then v = v_raw * recip ----------
                v_ps = psum_v.tile([P, D], fp32)
                for jc in range(4):
                    at_ps = psum.tile([P, P], bf16, tag="scratch")
                    nc.tensor.transpose(at_ps[:], e_bf[:, jc * P:(jc + 1) * P],
                                        ident_bf[:])
                    at_sb = work.tile([P, P], bf16, tag="at_sb")
                    nc.vector.tensor_copy(out=at_sb[:], in_=at_ps[:])
                    nc.tensor.matmul(v_ps[:], lhsT=at_sb[:], rhs=x_bf[:, jc],
                                     start=(jc == 0), stop=(jc == 3))
                v_sb = work.tile([P, D], fp32, tag="v")
                nc.scalar.activation(v_sb[:], v_ps[:], AF.Copy, bias=0.0,
                                     scale=recip[:])
                nc.sync.dma_start(out_ext.ap()[g * P:g * P + 64, :],
                                  v_sb[0:64, :])
                nc.sync.dma_start(out_ext.ap()[g * P + 64:(g + 1) * P, :],
                                   v_sb[64:P, :])

    return nc


_NC_CACHE = None


def make_in_maps(x, Wt, Wx, bh, Wa, ba):
    x = np.ascontiguousarray(np.asarray(x, dtype=np.float32))
    Wt = np.ascontiguousarray(
        np.asarray(Wt, dtype=np.float32).reshape(4, P, U).transpose(1, 0, 2))
    Wx = np.ascontiguousarray(
        np.asarray(Wx, dtype=np.float32).reshape(4, P, U).transpose(1, 0, 2))
    bh = np.ascontiguousarray(np.asarray(bh, dtype=np.float32))
    Wa = np.ascontiguousarray(np.asarray(Wa, dtype=np.float32)).reshape(U, 1)
    ba = np.ascontiguousarray(
        np.full((P, 1), np.asarray(ba, dtype=np.float32).reshape(()), np.float32))

    in_maps = []
    for c in range(N_CORES):
        b, ih = c // 2, c % 2
        # Attention sums over all keys j, so key order is irrelevant; roll the
        # rows so this core's 256 query rows are always rows 0..255 of its x.
        xb = x[b] if ih == 0 else np.roll(x[b], -IH, axis=0)
        in_maps.append({
            "x": np.ascontiguousarray(xb),
            "xT": np.ascontiguousarray(xb.T),
            "Wt": Wt, "Wx": Wx, "bh": bh, "Wa": Wa, "ba": ba,
        })
    return in_maps


def assemble_out(results):
    out = np.empty((B, L, D), dtype=np.float32)
    for c in range(N_CORES):
        b, ih = c // 2, c % 2
        out[b, ih * IH:(ih + 1) * IH, :] = results[c]["out"]
    return out


def kernel(x, mask, Wt, Wx, bh, Wa, ba):
    """Full inputs -> full output [B, L, D]. Shards over 8 NeuronCores."""
    global _NC_CACHE
    from concourse.bass_utils import run_bass_kernel_spmd

    if _NC_CACHE is None:
        _NC_CACHE = build_kernel()
        _NC_CACHE.finalize()
    nc = _NC_CACHE

    in_maps = make_in_maps(x, Wt, Wx, bh, Wa, ba)
    res = run_bass_kernel_spmd(nc, in_maps, core_ids=list(range(N_CORES)))
    return assemble_out(res.results)


if __name__ == "__main__":
    rng = np.random.default_rng(0)
    x = rng.standard_normal((B, L, D), dtype=np.float32)
    out = kernel(x, np.ones((B, L), bool),
                 rng.standard_normal((D, U), dtype=np.float32) * 0.05,
                 rng.standard_normal((D, U), dtype=np.float32) * 0.05,
                 np.zeros(U, np.float32),
                 rng.standard_normal((U, 1), dtype=np.float32) * 0.17,
                 np.zeros(1, np.float32))
    print(out.shape, out.dtype)


# revision 34
# speedup vs baseline: 1.0301x; 1.0254x over previous
"""Trainium2 Bass kernel for Bahdanau-style additive self-attention.

Reference computation (B=4, L=512, D=512, U=64):
    q = x @ Wt; k = x @ Wx                       [B, L, U]
    h = tanh(q[:, :, None, :] + k[:, None, :, :] + bh)       [B, L, L, U]
    e = exp(sigmoid(h . Wa + ba))                [B, L, L]
    a = e / (sum_j e + 1e-7)                     (mask is all-ones per spec)
    v = a @ x                                    [B, L, D]

Sharding: 8 cores, core c handles batch item b = c // 2 and query rows
[256 * (c % 2), ...+256).  Fully data-parallel, no collectives.  Host-side
layout prep (no arithmetic): rows of each core's x shard are rolled so its
query rows are rows 0..255 (attention sums over all keys, so key order is
irrelevant); x is also passed transposed (xT) so no on-device transpose of
x is needed; Wt/Wx are passed pre-chunked [128, 4, 64] for contiguous DMA;
ba is passed replicated [128, 1].

Per-core dataflow (ScalarE tanh throughput is the hard floor: 8.39M tanh
elements / 128 lanes / 1.2 GHz = 54.6 us):
  * qT = Wt^T x^T [64, 256] and kT-stacked = [Wx|Wx]^T x^T [128, 512] on
    PE (bf16 single-pass matmuls; fp32 matmuls cost two HI/LO passes).
  * K2 [128, 512] fp32 = kT stacked twice (2-query packing along the
    partition axis).  Qp [128, 128] fp32: column t = [qT[:, 2t] + bh ;
    qT[:, 2t+1] + bh].
  * main loop over blocks of G pairs (warmup [2,2,4,8] then G=12):
    VectorE tensor_scalar builds zb[:, j*512:...] = K2 + Qp[:, t] (the
    per-partition-scalar add); ONE ScalarE TANH over [128, G*512] fp32->
    bf16 amortizes the ~222-cycle ACT overhead; then G accumulating bf16
    matvecs with the sliding-window stationary WSLIDE[:, 128-2lt:256-2lt]
    (Wa at (rows 0:64, col 128) and (rows 64:128, col 129)) place pair
    lt's two score rows at PSUM partitions (2lt, 2lt+1): 64 matvecs build
    a dense [128, 512] fp32 score tile in one PSUM bank.
  * epilogue per score tile: sigmoid(z) = .5 + .5*tanh(z/2) ->
    w = tanh(.5 z + .5 ba); E = exp(.5 w + .5) -> bf16 with accum_out
    rowsums (tanh/exp live in one ACT table set: zero table switches);
    r = 1/(rowsum + eps) on VectorE reciprocal.
  * v = E @ x via PE-transposed bf16 E chunks against bf16 x chunks,
    fp32 PSUM accumulate; the 1/rowsum scale is folded into the ScalarE
    PSUM->SBUF copy (activation Copy with per-partition scale); DMA out.
"""

import os
import sys

import numpy as np

for _p in ("/root/.axon_site", "/root/.axon_site/_ro/trn_rl_repo",
           "/root/.axon_site/_ro/pypackages", "/opt/trn_rl_repo"):
    if os.path.isdir(_p) and _p not in sys.path:
        sys.path.append(_p)

B, L, D, U = 4, 512, 512, 64
P = 128
N_CORES = 8
IH = L // 2          # 256 query rows per core
NPAIR = IH // 2      # 128 packed query pairs per core
EPS = 1e-7


def build_kernel():
    import concourse.tile as tile
    from concourse import bacc, mybir
    from concourse.masks import make_identity

    fp32 = mybir.dt.float32
    bf16 = mybir.dt.bfloat16
    AF = mybir.ActivationFunctionType
    nc = bacc.Bacc()

    x_ext = nc.declare_dram_parameter("x", [L, D], bf16, isOutput=False)
    xt_ext = nc.declare_dram_parameter("xT", [D, L], bf16, isOutput=False)
    wt_ext = nc.declare_dram_parameter("Wt", [P, 4, U], bf16, isOutput=False)
    wx_ext = nc.declare_dram_parameter("Wx", [P, 4, U], bf16, isOutput=False)
    bh_ext = nc.declare_dram_parameter("bh", [U], fp32, isOutput=False)
    wa_ext = nc.declare_dram_parameter("Wa", [U, 1], fp32, isOutput=False)
    ba_ext = nc.declare_dram_parameter("ba", [P, 1], fp32, isOutput=False)
    out_ext = nc.declare_dram_parameter("out", [IH, D], fp32, isOutput=True)

    with tile.TileContext(nc) as tc:
        with (
            tc.tile_pool(name="const", bufs=1) as const,
            tc.tile_pool(name="work", bufs=3) as work,
            tc.tile_pool(name="tanh", bufs=3) as tanhp,
            tc.tile_pool(name="psum", bufs=4, space="PSUM") as psum,
            tc.tile_pool(name="psum_s", bufs=2, space="PSUM") as psum_s,
            tc.tile_pool(name="psum_v", bufs=2, space="PSUM") as psum_v,
        ):
            # ---- DMA enqueues first so transfers start ASAP -----------------
            # xT chunks on 3 queues (critical path: projections need them)
            xt_engines = [nc.sync, nc.scalar, nc.gpsimd, nc.sync]
            xT = []
            for dc in range(4):
                xtb = const.tile([P, L], bf16, tag=f"xtb{dc}")
                xt_engines[dc].dma_start(xtb[:], xt_ext.ap()[dc * P:(dc + 1) * P, :])
                xT.append(xtb)
            wx_bf = const.tile([P, 4, U], bf16)       # host pre-chunked [p, c, u]
            nc.scalar.dma_start(wx_bf[:], wx_ext.ap())
            wt_bf = const.tile([P, 4, U], bf16)
            nc.gpsimd.dma_start(wt_bf[:], wt_ext.ap())
            bh_sb = const.tile([U, 1], fp32)
            nc.sync.dma_start(bh_sb[:], bh_ext.ap()[:, None])
            ba_sb = const.tile([P, 1], fp32)          # ba replicated host-side
            nc.sync.dma_start(ba_sb[:], ba_ext.ap())
            wa_sb = const.tile([U, 1], fp32)
            nc.scalar.dma_start(wa_sb[:], wa_ext.ap())
            # x only feeds the v matmul (~60us in) -> load last
            x_bf = const.tile([P, 4, D], bf16)        # bf16 x for the v matmul
            for jc in range(4):
                xt_engines[jc].dma_start(x_bf[:, jc],
                                         x_ext.ap()[jc * P:(jc + 1) * P, :])

            # ---- constants; dummy tanh early hides ACT_TABLE_LOAD -----------
            half = const.tile([P, 1], fp32)
            nc.vector.memset(half[:], 0.5)
            dummy = const.tile([P, 1], fp32)
            nc.scalar.activation(dummy[:], half[:], AF.Tanh)
            ident_bf = const.tile([P, P], bf16)
            make_identity(nc, ident_bf)

            # doubled stationary [Wx | Wx]: kT comes out already stacked 2x
            wx2_bf = const.tile([P, 4, 2 * U], bf16)
            nc.vector.tensor_copy(out=wx2_bf[:, :, 0:U], in_=wx_bf[:])
            nc.vector.tensor_copy(out=wx2_bf[:, :, U:2 * U], in_=wx_bf[:])

            # ---- projections: qT first (qp overlaps the kT chain) -----------
            qT_ps = psum.tile([U, IH], fp32, tag="scratch")
            for dc in range(4):
                nc.tensor.matmul(qT_ps[:], lhsT=wt_bf[:, dc],
                                 rhs=xT[dc][:, 0:IH],
                                 start=(dc == 0), stop=(dc == 3))
            kT_ps = psum.tile([P, L], fp32, tag="scratch")
            for dc in range(4):
                nc.tensor.matmul(kT_ps[:], lhsT=wx2_bf[:, dc], rhs=xT[dc][:],
                                 start=(dc == 0), stop=(dc == 3))

            # Qp column t packs queries (2t, 2t+1) -> natural partition order
            qp = const.tile([P, NPAIR], fp32)
            qT_r = qT_ps.rearrange("u (t two) -> u two t", two=2)
            nc.vector.tensor_scalar(qp[0:U, :], qT_r[:, 0], bh_sb[:],
                                    None, mybir.AluOpType.add)
            nc.vector.tensor_scalar(qp[U:2 * U, :], qT_r[:, 1], bh_sb[:],
                                    None, mybir.AluOpType.add)
            k2 = const.tile([P, L], fp32)             # kT stacked twice
            nc.scalar.copy(k2[:], kT_ps[:])

            # ---- non-critical constants -------------------------------------
            wslide = const.tile([P, 2 * P], bf16)
            nc.vector.memset(wslide[:], 0.0)
            nc.vector.tensor_copy(out=wslide[0:U, P:P + 1], in_=wa_sb[:])
            nc.vector.tensor_copy(out=wslide[U:2 * U, P + 1:P + 2], in_=wa_sb[:])
            ba_half = const.tile([P, 1], fp32)
            nc.vector.tensor_scalar_mul(ba_half[:], ba_sb[:], 0.5)

            # ---- main loop: small warmup blocks, then G=16 steady ----------
            BLOCKS0 = [2, 2, 4, 8] + [12] * 4        # first group (fast ramp)
            BLOCKS1 = [12] * 4 + [8, 8]              # small last block: short tail
            for g in range(2):
                s_ps = psum_s.tile([P, L], fp32)
                lt = 0
                for gsz in (BLOCKS0 if g == 0 else BLOCKS1):
                    zb = work.tile([P, gsz * L], fp32, tag="zb")
                    for j in range(gsz):
                        t = g * 64 + lt + j
                        nc.vector.tensor_scalar_add(
                            zb[:, j * L:(j + 1) * L], k2[:], qp[:, t:t + 1])
                    tt = tanhp.tile([P, gsz * L], bf16)
                    nc.scalar.activation(tt[:], zb[:], AF.Tanh)
                    for j in range(gsz):
                        nc.tensor.matmul(
                            s_ps[:],
                            lhsT=wslide[:, P - 2 * (lt + j):2 * P - 2 * (lt + j)],
                            rhs=tt[:, j * L:(j + 1) * L],
                            start=(lt + j == 0), stop=(lt + j == 63))
                    lt += gsz

                # ---- epilogue: sigmoid via tanh, exp(+rowsum), normalize ---
                w_sb = work.tile([P, L], fp32, tag="w")
                nc.scalar.activation(w_sb[:], s_ps[:], AF.Tanh,
                                     bias=ba_half[:], scale=0.5)
                e_bf = work.tile([P, L], bf16, tag="e")
                rowsum = work.tile([P, 1], fp32, tag="rs")
                nc.scalar.activation(e_bf[:], w_sb[:], AF.Exp,
                                     bias=half[:], scale=0.5,
                                     accum_out=rowsum[:])
                recip = work.tile([P, 1], fp32, tag="rc")
                nc.vector.tensor_scalar_add(recip[:], rowsum[:], EPS)
                nc.vector.reciprocal(recip[:], recip[:])

                # ---- v_raw = E @ x (bf16), then v = v_raw * recip ----------
                v_ps = psum_v.tile([P, D], fp32)
                for jc in range(4):
                    at_ps = psum.tile([P, P], bf16, tag="scratch")
                    nc.tensor.transpose(at_ps[:], e_bf[:, jc * P:(jc + 1) * P],
                                        ident_bf[:])
                    at_sb = work.tile([P, P], bf16, tag="at_sb")
                    nc.vector.tensor_copy(out=at_sb[:], in_=at_ps[:])
                    nc.tensor.matmul(v_ps[:], lhsT=at_sb[:], rhs=x_bf[:, jc],
                                     start=(jc == 0), stop=(jc == 3))
                v_sb = work.tile([P, D], fp32, tag="v")
                nc.scalar.activation(v_sb[:], v_ps[:], AF.Copy, bias=0.0,
                                     scale=recip[:])
                nc.sync.dma_start(out_ext.ap()[g * P:g * P + 64, :],
                                  v_sb[0:64, :])
                nc.sync.dma_start(out_ext.ap()[g * P + 64:(g + 1) * P, :],
                                   v_sb[64:P, :])

    return nc


_NC_CACHE = None


def make_in_maps(x, Wt, Wx, bh, Wa, ba):
    import ml_dtypes
    bf16 = ml_dtypes.bfloat16
    # x/xT/Wt/Wx are consumed on-device only as bf16; casting host-side is
    # bit-identical to the device-side cast and halves the critical DMA bytes.
    x = np.asarray(x, dtype=np.float32).astype(bf16)
    Wt = np.ascontiguousarray(
        np.asarray(Wt, dtype=np.float32).reshape(4, P, U).transpose(1, 0, 2)
        .astype(bf16))
    Wx = np.ascontiguousarray(
        np.asarray(Wx, dtype=np.float32).reshape(4, P, U).transpose(1, 0, 2)
        .astype(bf16))
    bh = np.ascontiguousarray(np.asarray(bh, dtype=np.float32))
    Wa = np.ascontiguousarray(np.asarray(Wa, dtype=np.float32)).reshape(U, 1)
    ba = np.ascontiguousarray(
        np.full((P, 1), np.asarray(ba, dtype=np.float32).reshape(()), np.float32))

    in_maps = []
    for c in range(N_CORES):
        b, ih = c // 2, c % 2
        # Attention sums over all keys j, so key order is irrelevant; roll the
        # rows so this core's 256 query rows are always rows 0..255 of its x.
        xb = x[b] if ih == 0 else np.roll(x[b], -IH, axis=0)
        in_maps.append({
            "x": np.ascontiguousarray(xb),
            "xT": np.ascontiguousarray(xb.T),
            "Wt": Wt, "Wx": Wx, "bh": bh, "Wa": Wa, "ba": ba,
        })
    return in_maps


def assemble_out(results):
    out = np.empty((B, L, D), dtype=np.float32)
    for c in range(N_CORES):
        b, ih = c // 2, c % 2
        out[b, ih * IH:(ih + 1) * IH, :] = results[c]["out"]
    return out


def kernel(x, mask, Wt, Wx, bh, Wa, ba):
    """Full inputs -> full output [B, L, D]. Shards over 8 NeuronCores."""
    global _NC_CACHE
    from concourse.bass_utils import run_bass_kernel_spmd

    if _NC_CACHE is None:
        _NC_CACHE = build_kernel()
        _NC_CACHE.finalize()
    nc = _NC_CACHE

    in_maps = make_in_maps(x, Wt, Wx, bh, Wa, ba)
    res = run_bass_kernel_spmd(nc, in_maps, core_ids=list(range(N_CORES)))
    return assemble_out(res.results)


if __name__ == "__main__":
    rng = np.random.default_rng(0)
    x = rng.standard_normal((B, L, D), dtype=np.float32)
    out = kernel(x, np.ones((B, L), bool),
                 rng.standard_normal((D, U), dtype=np.float32) * 0.05,
                 rng.standard_normal((D, U), dtype=np.float32) * 0.05,
                 np.zeros(U, np.float32),
                 rng.standard_normal((U, 1), dtype=np.float32) * 0.17,
                 np.zeros(1, np.float32))
    print(out.shape, out.dtype)


# revision 35
# speedup vs baseline: 1.0644x; 1.0333x over previous
"""Trainium2 Bass kernel for Bahdanau-style additive self-attention.

Reference computation (B=4, L=512, D=512, U=64):
    q = x @ Wt; k = x @ Wx                       [B, L, U]
    h = tanh(q[:, :, None, :] + k[:, None, :, :] + bh)       [B, L, L, U]
    e = exp(sigmoid(h . Wa + ba))                [B, L, L]
    a = e / (sum_j e + 1e-7)                     (mask is all-ones per spec)
    v = a @ x                                    [B, L, D]

Sharding: 8 cores, core c handles batch item b = c // 2 and query rows
[256 * (c % 2), ...+256).  Fully data-parallel, no collectives.  Host-side
layout prep (no arithmetic): rows of each core's x shard are rolled so its
query rows are rows 0..255 (attention sums over all keys, so key order is
irrelevant); x is also passed transposed (xT) so no on-device transpose of
x is needed; Wt/Wx are passed pre-chunked [128, 4, 64] for contiguous DMA;
ba is passed replicated [128, 1].

Per-core dataflow (ScalarE tanh throughput is the hard floor: 8.39M tanh
elements / 128 lanes / 1.2 GHz = 54.6 us):
  * qT = Wt^T x^T [64, 256] and kT-stacked = [Wx|Wx]^T x^T [128, 512] on
    PE (bf16 single-pass matmuls; fp32 matmuls cost two HI/LO passes).
  * K2 [128, 512] fp32 = kT stacked twice (2-query packing along the
    partition axis).  Qp [128, 128] fp32: column t = [qT[:, 2t] + bh ;
    qT[:, 2t+1] + bh].
  * main loop over blocks of G pairs (warmup [2,2,4,8] then G=12):
    VectorE tensor_scalar builds zb[:, j*512:...] = K2 + Qp[:, t] (the
    per-partition-scalar add); ONE ScalarE TANH over [128, G*512] fp32->
    bf16 amortizes the ~222-cycle ACT overhead; then G accumulating bf16
    matvecs with the sliding-window stationary WSLIDE[:, 128-2lt:256-2lt]
    (Wa at (rows 0:64, col 128) and (rows 64:128, col 129)) place pair
    lt's two score rows at PSUM partitions (2lt, 2lt+1): 64 matvecs build
    a dense [128, 512] fp32 score tile in one PSUM bank.
  * epilogue per score tile: sigmoid(z) = .5 + .5*tanh(z/2) ->
    w = tanh(.5 z + .5 ba); E = exp(.5 w + .5) -> bf16 with accum_out
    rowsums (tanh/exp live in one ACT table set: zero table switches);
    r = 1/(rowsum + eps) on VectorE reciprocal.
  * v = E @ x via PE-transposed bf16 E chunks against bf16 x chunks,
    fp32 PSUM accumulate; the 1/rowsum scale is folded into the ScalarE
    PSUM->SBUF copy (activation Copy with per-partition scale); DMA out.
"""

import os
import sys

import numpy as np

for _p in ("/root/.axon_site", "/root/.axon_site/_ro/trn_rl_repo",
           "/root/.axon_site/_ro/pypackages", "/opt/trn_rl_repo"):
    if os.path.isdir(_p) and _p not in sys.path:
        sys.path.append(_p)

B, L, D, U = 4, 512, 512, 64
P = 128
N_CORES = 8
IH = L // 2          # 256 query rows per core
NPAIR = IH // 2      # 128 packed query pairs per core
EPS = 1e-7


def build_kernel():
    import concourse.tile as tile
    from concourse import bacc, mybir
    from concourse.masks import make_identity

    fp32 = mybir.dt.float32
    bf16 = mybir.dt.bfloat16
    AF = mybir.ActivationFunctionType
    nc = bacc.Bacc()

    x_ext = nc.declare_dram_parameter("x", [L, D], bf16, isOutput=False)
    xt_ext = nc.declare_dram_parameter("xT", [D, L], bf16, isOutput=False)
    wt_ext = nc.declare_dram_parameter("Wt", [P, 4, U], bf16, isOutput=False)
    wx_ext = nc.declare_dram_parameter("Wx", [P, 4, U], bf16, isOutput=False)
    bh_ext = nc.declare_dram_parameter("bh", [U], fp32, isOutput=False)
    wa_ext = nc.declare_dram_parameter("Wa", [U, 1], fp32, isOutput=False)
    ba_ext = nc.declare_dram_parameter("ba", [P, 1], fp32, isOutput=False)
    out_ext = nc.declare_dram_parameter("out", [IH, D], fp32, isOutput=True)

    with tile.TileContext(nc) as tc:
        with (
            tc.tile_pool(name="const", bufs=1) as const,
            tc.tile_pool(name="work", bufs=3) as work,
            tc.tile_pool(name="tanh", bufs=3) as tanhp,
            tc.tile_pool(name="psum", bufs=4, space="PSUM") as psum,
            tc.tile_pool(name="psum_s", bufs=2, space="PSUM") as psum_s,
            tc.tile_pool(name="psum_v", bufs=2, space="PSUM") as psum_v,
        ):
            # ---- DMA enqueues first so transfers start ASAP -----------------
            # xT chunks on 3 queues (critical path: projections need them)
            xt_engines = [nc.sync, nc.scalar, nc.gpsimd, nc.sync]
            xT = []
            for dc in range(4):
                xtb = const.tile([P, L], bf16, tag=f"xtb{dc}")
                xt_engines[dc].dma_start(xtb[:], xt_ext.ap()[dc * P:(dc + 1) * P, :])
                xT.append(xtb)
            wx_bf = const.tile([P, 4, U], bf16)       # host pre-chunked [p, c, u]
            nc.scalar.dma_start(wx_bf[:], wx_ext.ap())
            wt_bf = const.tile([P, 4, U], bf16)
            nc.gpsimd.dma_start(wt_bf[:], wt_ext.ap())
            bh_sb = const.tile([U, 1], fp32)
            nc.sync.dma_start(bh_sb[:], bh_ext.ap()[:, None])
            ba_sb = const.tile([P, 1], fp32)          # ba replicated host-side
            nc.sync.dma_start(ba_sb[:], ba_ext.ap())
            wa_sb = const.tile([U, 1], fp32)
            nc.scalar.dma_start(wa_sb[:], wa_ext.ap())
            # x only feeds the v matmul (~60us in) -> load last
            x_bf = const.tile([P, 4, D], bf16)        # bf16 x for the v matmul
            for jc in range(4):
                xt_engines[jc].dma_start(x_bf[:, jc],
                                         x_ext.ap()[jc * P:(jc + 1) * P, :])

            # ---- constants; dummy tanh early hides ACT_TABLE_LOAD -----------
            half = const.tile([P, 1], fp32)
            nc.vector.memset(half[:], 0.5)
            dummy = const.tile([P, 1], fp32)
            nc.scalar.activation(dummy[:], half[:], AF.Tanh)
            ident_bf = const.tile([P, P], bf16)
            make_identity(nc, ident_bf)

            # doubled stationary [Wx | Wx]: kT comes out already stacked 2x
            wx2_bf = const.tile([P, 4, 2 * U], bf16)
            nc.vector.tensor_copy(out=wx2_bf[:, :, 0:U], in_=wx_bf[:])
            nc.vector.tensor_copy(out=wx2_bf[:, :, U:2 * U], in_=wx_bf[:])

            # ---- projections: qT first (qp overlaps the kT chain) -----------
            qT_ps = psum.tile([U, IH], fp32, tag="scratch")
            for dc in range(4):
                nc.tensor.matmul(qT_ps[:], lhsT=wt_bf[:, dc],
                                 rhs=xT[dc][:, 0:IH],
                                 start=(dc == 0), stop=(dc == 3))
            kT_ps = psum.tile([P, L], fp32, tag="scratch")
            for dc in range(4):
                nc.tensor.matmul(kT_ps[:], lhsT=wx2_bf[:, dc], rhs=xT[dc][:],
                                 start=(dc == 0), stop=(dc == 3))

            # Qp column t packs queries (2t, 2t+1) -> natural partition order
            qp = const.tile([P, NPAIR], fp32)
            qT_r = qT_ps.rearrange("u (t two) -> u two t", two=2)
            nc.vector.tensor_scalar(qp[0:U, :], qT_r[:, 0], bh_sb[:],
                                    None, mybir.AluOpType.add)
            nc.vector.tensor_scalar(qp[U:2 * U, :], qT_r[:, 1], bh_sb[:],
                                    None, mybir.AluOpType.add)
            k2 = const.tile([P, L], fp32)             # kT stacked twice
            nc.scalar.copy(k2[:], kT_ps[:])

            # ---- non-critical constants -------------------------------------
            wslide = const.tile([P, 2 * P], bf16)
            nc.vector.memset(wslide[:], 0.0)
            nc.vector.tensor_copy(out=wslide[0:U, P:P + 1], in_=wa_sb[:])
            nc.vector.tensor_copy(out=wslide[U:2 * U, P + 1:P + 2], in_=wa_sb[:])
            ba_half = const.tile([P, 1], fp32)
            nc.vector.tensor_scalar_mul(ba_half[:], ba_sb[:], 0.5)

            # ---- main loop: small warmup blocks, then G=16 steady ----------
            BLOCKS0 = [2, 2, 4, 4, 8, 8, 12, 12, 12]  # smooth ramp (DVE pre-fill)
            BLOCKS1 = [12] * 4 + [8, 4, 4]            # small last block: short tail
            for g in range(2):
                s_ps = psum_s.tile([P, L], fp32)
                lt = 0
                for gsz in (BLOCKS0 if g == 0 else BLOCKS1):
                    zb = work.tile([P, gsz * L], fp32, tag="zb")
                    for j in range(gsz):
                        t = g * 64 + lt + j
                        nc.vector.tensor_scalar_add(
                            zb[:, j * L:(j + 1) * L], k2[:], qp[:, t:t + 1])
                    tt = tanhp.tile([P, gsz * L], bf16)
                    nc.scalar.activation(tt[:], zb[:], AF.Tanh)
                    for j in range(gsz):
                        nc.tensor.matmul(
                            s_ps[:],
                            lhsT=wslide[:, P - 2 * (lt + j):2 * P - 2 * (lt + j)],
                            rhs=tt[:, j * L:(j + 1) * L],
                            start=(lt + j == 0), stop=(lt + j == 63))
                    lt += gsz

                # ---- epilogue: sigmoid via tanh, exp(+rowsum), normalize ---
                w_sb = work.tile([P, L], fp32, tag="w")
                nc.scalar.activation(w_sb[:], s_ps[:], AF.Tanh,
                                     bias=ba_half[:], scale=0.5)
                e_bf = work.tile([P, L], bf16, tag="e")
                rowsum = work.tile([P, 1], fp32, tag="rs")
                nc.scalar.activation(e_bf[:], w_sb[:], AF.Exp,
                                     bias=half[:], scale=0.5,
                                     accum_out=rowsum[:])
                recip = work.tile([P, 1], fp32, tag="rc")
                nc.vector.tensor_scalar_add(recip[:], rowsum[:], EPS)
                nc.vector.reciprocal(recip[:], recip[:])

                # ---- v_raw = E @ x (bf16), then v = v_raw * recip ----------
                v_ps = psum_v.tile([P, D], fp32)
                for jc in range(4):
                    at_ps = psum.tile([P, P], bf16, tag="scratch")
                    nc.tensor.transpose(at_ps[:], e_bf[:, jc * P:(jc + 1) * P],
                                        ident_bf[:])
                    at_sb = work.tile([P, P], bf16, tag="at_sb")
                    nc.vector.tensor_copy(out=at_sb[:], in_=at_ps[:])
                    nc.tensor.matmul(v_ps[:], lhsT=at_sb[:], rhs=x_bf[:, jc],
                                     start=(jc == 0), stop=(jc == 3))
                v_sb = work.tile([P, D], fp32, tag="v")
                nc.scalar.activation(v_sb[:], v_ps[:], AF.Copy, bias=0.0,
                                     scale=recip[:])
                nc.sync.dma_start(out_ext.ap()[g * P:g * P + 64, :],
                                  v_sb[0:64, :])
                nc.sync.dma_start(out_ext.ap()[g * P + 64:(g + 1) * P, :],
                                   v_sb[64:P, :])

    return nc


_NC_CACHE = None


def make_in_maps(x, Wt, Wx, bh, Wa, ba):
    import ml_dtypes
    bf16 = ml_dtypes.bfloat16
    # x/xT/Wt/Wx are consumed on-device only as bf16; casting host-side is
    # bit-identical to the device-side cast and halves the critical DMA bytes.
    x = np.asarray(x, dtype=np.float32).astype(bf16)
    Wt = np.ascontiguousarray(
        np.asarray(Wt, dtype=np.float32).reshape(4, P, U).transpose(1, 0, 2)
        .astype(bf16))
    Wx = np.ascontiguousarray(
        np.asarray(Wx, dtype=np.float32).reshape(4, P, U).transpose(1, 0, 2)
        .astype(bf16))
    bh = np.ascontiguousarray(np.asarray(bh, dtype=np.float32))
    Wa = np.ascontiguousarray(np.asarray(Wa, dtype=np.float32)).reshape(U, 1)
    ba = np.ascontiguousarray(
        np.full((P, 1), np.asarray(ba, dtype=np.float32).reshape(()), np.float32))

    in_maps = []
    for c in range(N_CORES):
        b, ih = c // 2, c % 2
        # Attention sums over all keys j, so key order is irrelevant; roll the
        # rows so this core's 256 query rows are always rows 0..255 of its x.
        xb = x[b] if ih == 0 else np.roll(x[b], -IH, axis=0)
        in_maps.append({
            "x": np.ascontiguousarray(xb),
            "xT": np.ascontiguousarray(xb.T),
            "Wt": Wt, "Wx": Wx, "bh": bh, "Wa": Wa, "ba": ba,
        })
    return in_maps


def assemble_out(results):
    out = np.empty((B, L, D), dtype=np.float32)
    for c in range(N_CORES):
        b, ih = c // 2, c % 2
        out[b, ih * IH:(ih + 1) * IH, :] = results[c]["out"]
    return out


def kernel(x, mask, Wt, Wx, bh, Wa, ba):
    """Full inputs -> full output [B, L, D]. Shards over 8 NeuronCores."""
    global _NC_CACHE
    from concourse.bass_utils import run_bass_kernel_spmd

    if _NC_CACHE is None:
        _NC_CACHE = build_kernel()
        _NC_CACHE.finalize()
    nc = _NC_CACHE

    in_maps = make_in_maps(x, Wt, Wx, bh, Wa, ba)
    res = run_bass_kernel_spmd(nc, in_maps, core_ids=list(range(N_CORES)))
    return assemble_out(res.results)


if __name__ == "__main__":
    rng = np.random.default_rng(0)
    x = rng.standard_normal((B, L, D), dtype=np.float32)
    out = kernel(x, np.ones((B, L), bool),
                 rng.standard_normal((D, U), dtype=np.float32) * 0.05,
                 rng.standard_normal((D, U), dtype=np.float32) * 0.05,
                 np.zeros(U, np.float32),
                 rng.standard_normal((U, 1), dtype=np.float32) * 0.17,
                 np.zeros(1, np.float32))
    print(out.shape, out.dtype)
